# revision 11
# baseline (speedup 1.0000x reference)
"""Trainium2 Bass kernel for nn_ConditionalEstimation.

Computes, for full inputs:
    context[i] = sum_{j,k} a[i,j,k] * y[j] * z[k]          (i in [0, 384))
    scores[n]  = (x[n, :] @ context) / (context[0] + 1e-8)

Sharding across 8 NeuronCores (SPMD, one NEFF):
    - a is sharded along its leading i axis: core c owns a[c*48:(c+1)*48].
      Each core computes its 48-element slice of `context`, then an
      AllGather assembles the full 384-vector on every core.
    - x_candidates is sharded along N: core c owns rows [c*8192, (c+1)*8192)
      and computes those scores (pure data parallel).

Per-core device algorithm (v4):
    warm-up: a dummy AllGather triggered at t~0 absorbs the CC-stream entry
        barrier + ncfw startup (~50-70us incl. cross-core launch skew) while
        the a stream runs, so the real AllGather later starts immediately.
    phase 1: a streams as 24 two-group tiles [128, 2, 3, 384] (partition p
        holds j-rows 3p..3p+2 of both groups). Per group, three fp32r
        matmuls with y columns contract j into a [1, 384] PSUM row; a small
        DVE mul+reduce against z (reading PSUM directly) contracts k into
        ctxAll[0, g]. The Sync queue's first instruction is the first a
        DMA; constants load on the Scalar queue in parallel.
    AllGather(48 -> 384): bounce DMA on the Scalar HWDGE queue right after
        the last finisher; the collective overlaps the x stream.
    x prefetch: all xT chunk DMAs (and the DVE row-tile DMAs) are issued
        on Sync right after the a DMAs; every chunk has its own buffer.
    phase 2: post-AG setup is two small Scalar-queue DMAs plus a PE outer
        product that broadcasts the normalized context into PSUM. TensorE
        does the chunk matvecs (unnormalized fp32r weights; the 1/(den+eps)
        scale folds into the PSUM->SBUF copies, alternating Scalar/Vector);
        VectorE does the row-tile matvecs against the PSUM broadcast. All
        chunk outputs collect in one SBUF row and leave in a single DMA.
"""

import os
import sys

import numpy as np

sys.path.insert(0, "/opt/trn_rl_repo")

import concourse.bacc as bacc
import concourse.mybir as mybir
import concourse.tile as tile
from concourse.bass_utils import run_bass_kernel_spmd

N, D = 65536, 384
NC = 8
ISH = D // NC            # 48 context rows per core
XSH = N // NC            # 8192 candidate rows per core
EPS = 1e-8
FP = mybir.dt.float32
FPR = mybir.dt.float32r  # fp32 bits, reduced-precision PE compute (1 cyc/col)

NT = 16                  # phase-1 a tiles (3 i-groups each)
GPT = ISH // NT          # groups per tile (3)
ABUF = 4                 # a tile pool depth

PCH = 512                # PE path chunk width
NCH = XSH // PCH         # PE path chunks (16) — all rows via TensorE

_CACHE = {}
LAST_RESULT = None  # BassKernelResults of the most recent run (for test harness)


def _build():
    if "nc" in _CACHE:
        return _CACHE["nc"]

    from concourse.tile import add_dep_helper

    nc = bacc.Bacc("TRN2", target_bir_lowering=False, debug=False, num_devices=NC)
    Alu = mybir.AluOpType
    Act = mybir.ActivationFunctionType

    a_d = nc.dram_tensor("a_sh", [NT, GPT, D, D], FPR, kind="ExternalInput")
    # chunk-major transposed x: [chunk, d, q] so each chunk DMA is contiguous
    xp_d = nc.dram_tensor("xT_pe", [NCH, D, PCH], FPR, kind="ExternalInput")
    y_d = nc.dram_tensor("y", [D], FPR, kind="ExternalInput")
    z_d = nc.dram_tensor("z", [D], FP, kind="ExternalInput")
    o_d = nc.dram_tensor("scores_sh", [XSH], FP, kind="ExternalOutput")

    with tile.TileContext(nc) as tc:
        with (
            tc.tile_pool(name="const", bufs=1) as cst,
            tc.tile_pool(name="a", bufs=ABUF) as a_pool,
            tc.tile_pool(name="xtp", bufs=NCH) as xt_pool,
            tc.tile_pool(name="scr", bufs=4) as scr_pool,
            tc.tile_pool(name="acc", bufs=1) as acc_pool,
            tc.tile_pool(name="ps", bufs=6, space="PSUM") as ps_pool,
            tc.tile_pool(name="psb", bufs=1, space="PSUM") as psb_pool,
            tc.tile_pool(name="dram", bufs=1, space="DRAM") as dram_pool,
        ):
            # --- warm-up collective (see module docstring) ---
            dummy = cst.tile([1, 8], FP)
            nc.vector.memset(dummy[:], 0.0)
            cc_din = dram_pool.tile([8], FP)
            cc_dout = dram_pool.tile([8 * NC], FP)
            nc.scalar.dma_start(cc_din[:], dummy[:])
            nc.gpsimd.collective_compute(
                "AllGather",
                Alu.bypass,
                replica_groups=[list(range(NC))],
                ins=[cc_din.opt()],
                outs=[cc_dout.opt()],
            )

            # --- constants (Scalar HWDGE, parallel to the a stream) ---
            # y permuted to match the a-tile layout: y3p[p, s] = y[3p + s]
            y3p = cst.tile([128, 3], FPR)
            nc.scalar.dma_start(y3p[:], y_d.ap().rearrange("(p s) -> p s", s=3))
            zrow = cst.tile([1, D], FP)      # z on partition 0 (finisher mul)
            nc.scalar.dma_start(zrow[:], z_d.ap().unsqueeze(0))
            ones11 = cst.tile([1, 1], FP)
            nc.vector.memset(ones11[:], 1.0)

            # --- phase 1: PE j-contraction + DVE k-finisher per group ---
            ctxAll = acc_pool.tile([1, ISH], FP)
            at_last = None
            for t in range(NT):
                at = a_pool.tile([128, GPT, 3, D], FPR, tag="a")
                eng = nc.sync if t % 2 == 0 else nc.scalar
                eng.dma_start(
                    at[:], a_d.ap()[t].rearrange("g (p s) k -> p g s k", s=3)
                )
                at_last = at
                for g2 in range(GPT):
                    g = GPT * t + g2
                    ups = ps_pool.tile([1, D], FP, tag="ps")
                    for s in range(3):
                        nc.tensor.matmul(
                            ups[:], y3p[:, s:s + 1], at[:, g2, s, :],
                            start=(s == 0), stop=(s == 2),
                        )
                    scr = scr_pool.tile([1, D], FP, tag="fin")
                    nc.vector.tensor_mul(scr[:], ups[:], zrow[:])
                    nc.vector.tensor_reduce(
                        ctxAll[:, g:g + 1], scr[:],
                        axis=mybir.AxisListType.X, op=Alu.add,
                    )

            # --- AllGather the context slices (bounce DMA on Scalar HWDGE) ---
            cc_in = dram_pool.tile([ISH], FP)
            cc_out = dram_pool.tile([D], FP)
            nc.scalar.dma_start(cc_in[:], ctxAll[:])
            nc.gpsimd.collective_compute(
                "AllGather",
                Alu.bypass,
                replica_groups=[list(range(NC))],
                ins=[cc_in.opt()],
                outs=[cc_out.opt()],
            )

            # --- PE warm-keepers: ~40 dependency-free matmuls right after
            # phase 1 keep the PE clock high through the AllGather window
            # (~13us << min observed trigger->AG-end of ~26us, so they never
            # delay phase 2). Results go to a scratch PSUM tile nothing reads.
            for w in range(20):
                wps = ps_pool.tile([1, D], FP, tag="ps")
                for s2 in range(2):
                    nc.tensor.matmul(
                        wps[:], y3p[:, s2:s2 + 1], at_last[:, 0, s2, :],
                        start=(s2 == 0), stop=(s2 == 1),
                    )

            # --- x prefetch: issued on Sync right after the a DMAs, BEFORE
            # anything that waits on the AllGather; every chunk has its own
            # buffer, so all of x streams during the collective.
            xcs = []
            x_dmas = []
            for c in range(NCH):
                xc = xt_pool.tile([128, 3, PCH], FPR)
                # [p, s, q] = xT chunk row 128s+p (matches the ctxT weight
                # layout): three 2KB blocks per partition
                dma = nc.sync.dma_start(
                    xc[:], xp_d.ap()[c].rearrange("(s p) q -> p s q", p=128)
                )
                xcs.append(xc)
                x_dmas.append(dma)
            last_x = x_dmas[-1]

            # --- post-AG context setup: one contiguous row read, then
            # on-chip transposes (outer products with a ones scalar) build
            # the [128, 3] weight layout without a strided HBM hop.
            ctxrow = cst.tile([1, D], FP)    # full context on partition 0
            nc.scalar.dma_start(ctxrow[:], cc_out[:].unsqueeze(0))
            rec1 = cst.tile([1, 1], FP)      # 1/(context[0]+eps)
            nc.vector.tensor_scalar_add(rec1[:], ctxrow[:, 0:1], EPS)
            nc.vector.reciprocal(rec1[:], rec1[:])
            ctxT = psb_pool.tile([128, 3], FP)
            for s2 in range(3):
                # ctxT[p, s2] = ctxrow[0, 128*s2 + p]
                nc.tensor.matmul(
                    ctxT[:, s2:s2 + 1], ctxrow[:, 128 * s2:128 * (s2 + 1)],
                    ones11[:], start=True, stop=True,
                )
            ctx3r = cst.tile([128, 3], FPR)  # fp32r weights (unnormalized)
            nc.vector.tensor_scalar_mul(ctx3r[:], ctxT[:], 1.0)

            # --- phase 2 (TensorE): all 8192 rows via x^T chunks ---
            so_all = acc_pool.tile([1, XSH], FP)
            for c in range(NCH):
                sps = ps_pool.tile([1, PCH], FP, tag="ps")
                for s in range(3):
                    nc.tensor.matmul(
                        sps[:], ctx3r[:, s:s + 1], xcs[c][:, s, :],
                        start=(s == 0), stop=(s == 2),
                    )
                dst = so_all[:, c * PCH:(c + 1) * PCH]
                nc.scalar.activation(dst, sps[:], Act.Copy, scale=rec1[:])
            od = nc.sync.dma_start(o_d.ap(), so_all[:])
            add_dep_helper(od.ins, last_x.ins, sync=False,
                           reason="keep output DMAs after x prefetch issues")

    nc.compile()
    _CACHE["nc"] = nc
    return nc


def make_in_maps(x_candidates, y, z, a):
    x_candidates = np.ascontiguousarray(x_candidates, dtype=np.float32)
    y = np.ascontiguousarray(y, dtype=np.float32)
    z = np.ascontiguousarray(z, dtype=np.float32)
    a = np.ascontiguousarray(a, dtype=np.float32)
    in_maps = []
    for c in range(NC):
        x_sh = x_candidates[c * XSH:(c + 1) * XSH]
        xt = np.ascontiguousarray(
            x_sh.T.reshape(D, NCH, PCH).transpose(1, 0, 2)
        )
        a_sh = a[c * ISH:(c + 1) * ISH].reshape(NT, GPT, D, D)
        in_maps.append({
            "a_sh": a_sh,
            "xT_pe": xt,
            "y": y,
            "z": z,
        })
    return in_maps


def kernel(x_candidates, y, z, a):
    global LAST_RESULT
    nc = _build()
    in_maps = make_in_maps(x_candidates, y, z, a)

    trace = os.environ.get("CC_KERNEL_TRACE", "0") == "1"
    try:
        res = run_bass_kernel_spmd(nc, in_maps, core_ids=list(range(NC)), trace=trace)
    except Exception:
        if not trace:
            raise
        # Trace post-processing can fail in minimal containers; results
        # are what matter — retry without tracing.
        res = run_bass_kernel_spmd(nc, in_maps, core_ids=list(range(NC)), trace=False)
    LAST_RESULT = res
    out = np.concatenate([res.results[c]["scores_sh"] for c in range(NC)])
    return np.ascontiguousarray(out, dtype=np.float32)
